# revision 2
# baseline (speedup 1.0000x reference)
"""Trainium2 Bass kernel for CrossAttention (B=4, L=S=2048, DIM=1024, H=16, hd=64).

Sharding: data-parallel over (batch, L-half): core c handles batch c//2,
query rows [(c%2)*1024, (c%2+1)*1024).  Each core computes the QKV
projections for its slice (K/V duplicated within a batch pair), per-head
RMSNorm, masked softmax attention, and the output projection.

Device layout is feature-major ("transposed"): activations live as
[dim, tokens] so every matmul contraction dim is on SBUF partitions with
no on-device transposes.  The host pre-transposes q/kv and casts to bf16.

Softmax: after RMS norm |score| <= 8, so no running max is needed.  exp
runs on ACT with a fused per-partition scale (k-norm rsqrt / 8) and bias
(padding mask, -1e5 -> exp == 0).  The denominator comes from a 65th
"ones" column appended to V; the division is applied to o^T (broadcast
via a rank-1 PE matmul) before proj_o.
"""

import sys

if "/opt/trn_rl_repo" not in sys.path:
    sys.path.insert(0, "/opt/trn_rl_repo")

import numpy as np
import ml_dtypes

import concourse.bass as bass
import concourse.bacc as bacc
import concourse.tile as tile
from concourse import mybir
from concourse.bass_utils import run_bass_kernel_spmd

BF16 = ml_dtypes.bfloat16

B, L, S, DIM = 4, 2048, 2048, 1024
H, HD = 16, 64
N_CORES = 8
LC = L // 2          # query rows per core
KC = DIM // 128      # 128-partition chunks of DIM
EPS = 1e-5
MASK_BIAS = -1.0e5   # exp(-1e5) == 0 in fp32

TRACE = False        # set by test.py for profiling
LAST_RESULT = {}     # exec_time_ns etc. for test.py

_CACHE = {}


def _build(n_sc):
    """Build the SPMD Bass program; n_sc = number of 128-wide kv chunks."""
    fp32 = mybir.dt.float32
    bf16 = mybir.dt.bfloat16
    AF = mybir.ActivationFunctionType

    nc = bacc.Bacc("TRN2", target_bir_lowering=False, debug=False,
                   num_devices=N_CORES)

    qT_d = nc.dram_tensor("qT", [DIM, LC], bf16, kind="ExternalInput")
    kvT_d = nc.dram_tensor("kvT", [DIM, S], bf16, kind="ExternalInput")
    wq_d = nc.dram_tensor("wq", [DIM, DIM], bf16, kind="ExternalInput")
    wk_d = nc.dram_tensor("wk", [DIM, DIM], bf16, kind="ExternalInput")
    wv_d = nc.dram_tensor("wv", [DIM, DIM], bf16, kind="ExternalInput")
    wo_d = nc.dram_tensor("wo", [DIM, DIM], bf16, kind="ExternalInput")
    mask_d = nc.dram_tensor("mask", [128, 16], fp32, kind="ExternalInput")
    qw_d = nc.dram_tensor("qw", [128, 1], fp32, kind="ExternalInput")
    kw_d = nc.dram_tensor("kw", [128, 1], fp32, kind="ExternalInput")
    ind_d = nc.dram_tensor("ind", [KC, 128, 16], bf16, kind="ExternalInput")
    ind2_d = nc.dram_tensor("ind2", [KC, 16, 128], fp32, kind="ExternalInput")
    eye_d = nc.dram_tensor("eye16", [16, 16], fp32, kind="ExternalInput")
    out_d = nc.dram_tensor("out", [LC, DIM], fp32, kind="ExternalOutput")

    n_sg = (n_sc + 3) // 4           # 512-wide kv groups for the K projection
    SKC = n_sg * 4                   # padded chunk count for K-side tiles

    with tile.TileContext(nc) as tc:
        with (
            tc.tile_pool(name="wp", bufs=3 * KC) as wp,          # wq wk wv (+wo reuse)
            tc.tile_pool(name="qtp", bufs=KC) as qtp,            # qT, later oT
            tc.tile_pool(name="kvp", bufs=KC) as kvp,            # kvT, later exp/tmp
            tc.tile_pool(name="qhp", bufs=KC) as qhp,            # qhT
            tc.tile_pool(name="khp", bufs=KC) as khp,            # khT
            tc.tile_pool(name="vp", bufs=n_sc) as vp,            # vh_aug
            tc.tile_pool(name="sp", bufs=1) as sp,               # constants
            tc.tile_pool(name="skp", bufs=SKC) as skp,           # skT per chunk
            tc.tile_pool(name="tp", bufs=4) as tp,               # f32 temps
            tc.tile_pool(name="pa", bufs=2, space="PSUM") as pa,     # proj/score psum
            tc.tile_pool(name="po", bufs=4, space="PSUM") as po,     # accum/bc psum
        ):
            # ---------------- constants / inputs ----------------
            wq_sb, wk_sb, wv_sb, kvt_sb, qt_sb = [], [], [], [], []
            for k in range(KC):
                w1 = wp.tile([128, DIM], bf16, name=f"wq{k}", tag="w")
                nc.sync.dma_start(out=w1, in_=wq_d[k * 128:(k + 1) * 128, :])
                wq_sb.append(w1)
            for k in range(KC):
                w2 = wp.tile([128, DIM], bf16, name=f"wk{k}", tag="w")
                nc.sync.dma_start(out=w2, in_=wk_d[k * 128:(k + 1) * 128, :])
                wk_sb.append(w2)
            for k in range(KC):
                w3 = wp.tile([128, DIM], bf16, name=f"wv{k}", tag="w")
                nc.sync.dma_start(out=w3, in_=wv_d[k * 128:(k + 1) * 128, :])
                wv_sb.append(w3)
            for k in range(KC):
                qt = qtp.tile([128, LC], bf16, name=f"qt{k}", tag="qt")
                nc.sync.dma_start(out=qt, in_=qT_d[k * 128:(k + 1) * 128, :])
                qt_sb.append(qt)
            for k in range(KC):
                kvt = kvp.tile([128, S], bf16, name=f"kvt{k}", tag="kv")
                nc.sync.dma_start(out=kvt, in_=kvT_d[k * 128:(k + 1) * 128, :])
                kvt_sb.append(kvt)

            mask_sb = sp.tile([128, 16], fp32, name="mask")
            nc.sync.dma_start(out=mask_sb, in_=mask_d[:, :])
            qw_sb = sp.tile([128, 1], fp32, name="qw")
            nc.sync.dma_start(out=qw_sb, in_=qw_d[:, :])
            kw_sb = sp.tile([128, 1], fp32, name="kw")
            nc.sync.dma_start(out=kw_sb, in_=kw_d[:, :])
            eye_sb = sp.tile([16, 16], fp32, name="eye16")
            nc.sync.dma_start(out=eye_sb, in_=eye_d[:, :])
            ind_sb, ind2_sb = [], []
            for k in range(KC):
                i1 = sp.tile([128, 16], bf16, name=f"ind{k}")
                nc.sync.dma_start(out=i1, in_=ind_d[k, :, :])
                ind_sb.append(i1)
                i2 = sp.tile([16, 128], fp32, name=f"ind2{k}")
                nc.sync.dma_start(out=i2, in_=ind2_d[k, :, :])
                ind2_sb.append(i2)
            ones_sb = sp.tile([128, 64], fp32, name="ones")
            nc.vector.memset(ones_sb, 1.0)
            epsq_sb = sp.tile([16, 1], fp32, name="epsq")
            nc.vector.memset(epsq_sb, EPS)
            epsk_sb = sp.tile([16, 1], fp32, name="epsk")
            nc.vector.memset(epsk_sb, 64.0 * EPS)

            # ---------------- Q projection + q RMS stats ----------------
            qh_sb = [qhp.tile([128, LC], bf16, name=f"qh{m}", tag="qh")
                     for m in range(KC)]
            sumsq_q = [po.tile([16, 512], fp32, name=f"ssq{j}", tag="po")
                       for j in range(2)]
            for m in range(KC):
                for j in range(2):
                    ps = pa.tile([128, 1024], fp32, name="proj_ps", tag="pa")
                    for k in range(KC):
                        nc.tensor.matmul(
                            ps[:, :512],
                            lhsT=wq_sb[k][:, m * 128:(m + 1) * 128],
                            rhs=qt_sb[k][:, j * 512:(j + 1) * 512],
                            start=(k == 0), stop=(k == KC - 1))
                    nc.vector.tensor_scalar_mul(
                        qh_sb[m][:, j * 512:(j + 1) * 512], ps[:, :512], qw_sb)
                    qsq = kvp.tile([128, 1024], bf16, name="sqt", tag="sq",
                                   bufs=3)
                    nc.scalar.activation(qsq[:, :512], ps[:, :512], AF.Square)
                    nc.tensor.matmul(
                        sumsq_q[j][:, :], lhsT=ind_sb[m], rhs=qsq[:, :512],
                        start=(m == 0), stop=(m == KC - 1))
            # sq = 1/sqrt(mean + eps), broadcast to rows, fold into qhT
            sq_sb = []
            for j in range(2):
                sqr = tp.tile([16, 512], fp32, name=f"sqr{j}", tag="small16")
                nc.scalar.activation(sqr, sumsq_q[j][:, :], AF.Sqrt,
                                     scale=1.0 / HD, bias=epsq_sb)
                sqv = tp.tile([16, 512], fp32, name=f"sqv{j}", tag="small16b")
                nc.vector.reciprocal(out=sqv, in_=sqr)
                sq_sb.append(sqv)
            for m in range(KC):
                for j in range(2):
                    bc = po.tile([128, 512], fp32, name="qbc", tag="po")
                    nc.tensor.matmul(bc, lhsT=ind2_sb[m], rhs=sq_sb[j],
                                     start=True, stop=True)
                    nc.vector.tensor_mul(
                        qh_sb[m][:, j * 512:(j + 1) * 512],
                        qh_sb[m][:, j * 512:(j + 1) * 512], bc)

            # ---------------- K projection + skT ----------------
            kh_sb = [khp.tile([128, S], bf16, name=f"kh{m}", tag="kh")
                     for m in range(KC)]
            skT_sb = [skp.tile([128, 16], fp32, name=f"skT{i}", tag="sk")
                      for i in range(SKC)]
            for sg in range(n_sg):
                ssk = po.tile([16, 512], fp32, name="ssk", tag="po")
                for m in range(KC):
                    ps = pa.tile([128, 1024], fp32, name="proj_ps", tag="pa")
                    for k in range(KC):
                        nc.tensor.matmul(
                            ps[:, :512],
                            lhsT=wk_sb[k][:, m * 128:(m + 1) * 128],
                            rhs=kvt_sb[k][:, sg * 512:(sg + 1) * 512],
                            start=(k == 0), stop=(k == KC - 1))
                    nc.vector.tensor_scalar_mul(
                        kh_sb[m][:, sg * 512:(sg + 1) * 512], ps[:, :512],
                        kw_sb)
                    ksq = kvp.tile([128, 1024], bf16, name="sqt", tag="sq",
                                   bufs=3)
                    nc.scalar.activation(ksq[:, :512], ps[:, :512], AF.Square)
                    nc.tensor.matmul(
                        ssk[:, :], lhsT=ind_sb[m], rhs=ksq[:, :512],
                        start=(m == 0), stop=(m == KC - 1))
                # 8*sqrt(mean+eps) = sqrt(sumsq + 64 eps); recip -> sk/8
                skr = tp.tile([16, 512], fp32, name="skr", tag="small16")
                nc.scalar.activation(skr, ssk[:, :], AF.Sqrt,
                                     scale=1.0, bias=epsk_sb)
                for t in range(4):
                    tpp = po.tile([128, 16], fp32, name="sktp", tag="po")
                    nc.tensor.transpose(tpp, skr[:, t * 128:(t + 1) * 128],
                                        eye_sb)
                    nc.vector.reciprocal(out=skT_sb[sg * 4 + t], in_=tpp)

            # ---------------- V projection (token-major, ones-augmented) ----
            va_sb = []
            for i in range(n_sc):
                va = vp.tile([128, H * 65], bf16, name=f"va{i}", tag="va")
                ones_cols = bass.AP(tensor=va.tensor, offset=va.offset + 64,
                                    ap=[list(va.ap[0]), [65, H], [1, 1]])
                nc.vector.memset(ones_cols, 1.0)
                for jn in range(2):
                    ps = pa.tile([128, 1024], fp32, name="proj_ps", tag="pa")
                    for k in range(KC):
                        nc.tensor.matmul(
                            ps[:, :512],
                            lhsT=kvt_sb[k][:, i * 128:(i + 1) * 128],
                            rhs=wv_sb[k][:, jn * 512:(jn + 1) * 512],
                            start=(k == 0), stop=(k == KC - 1))
                    dst = bass.AP(tensor=va.tensor,
                                  offset=va.offset + 65 * 8 * jn,
                                  ap=[list(va.ap[0]), [65, 8], [1, 64]])
                    nc.vector.tensor_copy(
                        dst, ps[:, :512].rearrange("p (h d) -> p h d", h=8))
                va_sb.append(va)

            # ---------------- attention ----------------
            oT_sb = [qtp.tile([128, LC], bf16, name=f"oT{m}", tag="qt")
                     for m in range(KC)]
            for h in range(H):
                m, poff = h // 2, (h % 2) * 64
                pv = [po.tile([128, 512], fp32, name=f"pv{j}", tag="po")
                      for j in range(2)]
                for i in range(n_sc):
                    scp = pa.tile([128, 1024], fp32, name="sc_ps", tag="pa")
                    for j in range(2):
                        nc.tensor.matmul(
                            scp[:, j * 512:(j + 1) * 512],
                            lhsT=kh_sb[m][poff:poff + 64,
                                          i * 128:(i + 1) * 128],
                            rhs=qh_sb[m][poff:poff + 64,
                                         j * 512:(j + 1) * 512],
                            start=True, stop=True)
                    ex = kvp.tile([128, 1024], bf16, name="ex", tag="sq",
                                  bufs=3)
                    nc.scalar.activation(ex, scp, AF.Exp,
                                         scale=skT_sb[i][:, h:h + 1],
                                         bias=mask_sb[:, i:i + 1])
                    for j in range(2):
                        nc.tensor.matmul(
                            pv[j][:65, :],
                            lhsT=va_sb[i][:, h * 65:(h + 1) * 65],
                            rhs=ex[:, j * 512:(j + 1) * 512],
                            start=(i == 0), stop=(i == n_sc - 1))
                for j in range(2):
                    rec = tp.tile([128, 512], fp32, name="rec", tag="rec")
                    nc.vector.reciprocal(out=rec[64:65, :], in_=pv[j][64:65, :])
                    bcp = po.tile([64, 512], fp32, name="obc", tag="po")
                    nc.tensor.matmul(bcp, lhsT=ones_sb[64:65, 0:64],
                                     rhs=rec[64:65, :], start=True, stop=True)
                    bcs = tp.tile([64, 512], fp32, name="bcs", tag="bcs")
                    nc.vector.tensor_copy(bcs, bcp)
                    nc.vector.tensor_mul(
                        oT_sb[m][poff:poff + 64, j * 512:(j + 1) * 512],
                        pv[j][0:64, :], bcs)

            # ---------------- output projection ----------------
            wo_sb = []
            for k in range(KC):
                w4 = wp.tile([128, DIM], bf16, name=f"wo{k}", tag="w")
                nc.sync.dma_start(out=w4, in_=wo_d[k * 128:(k + 1) * 128, :])
                wo_sb.append(w4)
            for lc in range(KC):
                for jn in range(2):
                    ps = pa.tile([128, 1024], fp32, name="proj_ps", tag="pa")
                    for k in range(KC):
                        nc.tensor.matmul(
                            ps[:, :512],
                            lhsT=oT_sb[k][:, lc * 128:(lc + 1) * 128],
                            rhs=wo_sb[k][:, jn * 512:(jn + 1) * 512],
                            start=(k == 0), stop=(k == KC - 1))
                    osb = tp.tile([128, 512], fp32, name="osb", tag="rec")
                    nc.vector.tensor_copy(osb, ps[:, :512])
                    nc.sync.dma_start(
                        out=out_d[lc * 128:(lc + 1) * 128,
                                  jn * 512:(jn + 1) * 512],
                        in_=osb)
    nc.compile()
    return nc


def kernel(**inputs):
    q = np.asarray(inputs["q"], dtype=np.float32)
    kv = np.asarray(inputs["kv"], dtype=np.float32)
    seqlens = np.asarray(inputs["x_seqlens"], dtype=np.int32)
    Wq = np.asarray(inputs["Wq"], dtype=np.float32)
    Wk = np.asarray(inputs["Wk"], dtype=np.float32)
    Wv = np.asarray(inputs["Wv"], dtype=np.float32)
    Wo = np.asarray(inputs["Wo"], dtype=np.float32)
    qnw = np.asarray(inputs["q_norm_w"], dtype=np.float32)
    knw = np.asarray(inputs["k_norm_w"], dtype=np.float32)

    n_sc = max(1, int(-(-int(seqlens.max()) // 128)))
    if n_sc not in _CACHE:
        _CACHE[n_sc] = _build(n_sc)
    nc = _CACHE[n_sc]

    wq_b = np.ascontiguousarray(Wq).astype(BF16)
    wk_b = np.ascontiguousarray(Wk).astype(BF16)
    wv_b = np.ascontiguousarray(Wv).astype(BF16)
    wo_b = np.ascontiguousarray(Wo).astype(BF16)
    qw = np.tile(qnw, 2).reshape(128, 1)
    kw = np.tile(knw, 2).reshape(128, 1)
    ind = np.zeros((KC, 128, 16), np.float32)
    ind2 = np.zeros((KC, 16, 128), np.float32)
    p = np.arange(128)
    for c in range(KC):
        ind[c, p, 2 * c + p // 64] = 1.0
        ind2[c, 2 * c + p // 64, p] = 1.0
    ind = ind.astype(BF16)
    eye16 = np.eye(16, dtype=np.float32)

    in_maps = []
    for c in range(N_CORES):
        b, half = c // 2, c % 2
        qT = np.ascontiguousarray(
            q[b, half * LC:(half + 1) * LC, :].T).astype(BF16)
        kvT = np.ascontiguousarray(kv[b].T).astype(BF16)
        sl = int(seqlens[b])
        mask = np.where(np.arange(S) < sl, 0.0, MASK_BIAS).astype(np.float32)
        mask = np.ascontiguousarray(mask.reshape(16, 128).T)
        in_maps.append({
            "qT": qT, "kvT": kvT, "wq": wq_b, "wk": wk_b, "wv": wv_b,
            "wo": wo_b, "mask": mask, "qw": qw, "kw": kw, "ind": ind,
            "ind2": ind2, "eye16": eye16,
        })

    res = run_bass_kernel_spmd(nc, in_maps, list(range(N_CORES)),
                               trace=TRACE)
    LAST_RESULT["exec_time_ns"] = res.exec_time_ns
    LAST_RESULT["profile"] = res.profile_json

    out = np.empty((B, L, DIM), np.float32)
    for c in range(N_CORES):
        b, half = c // 2, c % 2
        out[b, half * LC:(half + 1) * LC, :] = res.results[c]["out"]
    return out


# revision 6
# speedup vs baseline: 1.3887x; 1.3887x over previous
"""Trainium2 Bass kernel for CrossAttention (B=4, L=S=2048, DIM=1024, H=16, hd=64).

Sharding: data-parallel over (batch, L-half): core c handles batch c//2,
query rows [(c%2)*1024, (c%2+1)*1024).  Each core computes the QKV
projections for its slice (K/V duplicated within a batch pair), per-head
RMSNorm, masked softmax attention, and the output projection.

Device layout is feature-major ("transposed"): activations live as
[dim, tokens] so every matmul contraction dim is on SBUF partitions with
no on-device transposes.  The host pre-transposes q/kv and casts to bf16.

Softmax: after RMS norm |score| <= 8, so no running max is needed.  exp
runs on ACT with a fused per-partition scale (k-norm rsqrt / 8) and bias
(padding mask, -1e5 -> exp == 0).  The denominator comes from a 65th
"ones" column appended to V; the division is deferred: o^T is stored
unnormalized, denominators are collected per head, and one batched
reciprocal + rank-1 PE broadcast normalizes o^T before proj_o.

Perf notes (v2): score matmuls for a head pair run concurrently in the
PE array via tile_position row packing (K=64 each); PV and sumsq
matmuls are emitted one iteration late so the PE queue head never waits
on ACT; normalization is off the per-head critical path entirely.
"""

import sys

if "/opt/trn_rl_repo" not in sys.path:
    sys.path.insert(0, "/opt/trn_rl_repo")

import numpy as np
import ml_dtypes

import concourse.bass as bass
import concourse.bacc as bacc
import concourse.tile as tile
from concourse import mybir
from concourse.bass_utils import run_bass_kernel_spmd

BF16 = ml_dtypes.bfloat16

B, L, S, DIM = 4, 2048, 2048, 1024
H, HD = 16, 64
N_CORES = 8
LC = L // 2          # query rows per core
KC = DIM // 128      # 128-partition chunks of DIM
EPS = 1e-5
MASK_BIAS = -1.0e5   # exp(-1e5) == 0 in fp32

TRACE = False        # set by test.py for profiling
LAST_RESULT = {}     # exec_time_ns etc. for test.py

_CACHE = {}


def _build(n_sc):
    """Build the SPMD Bass program; n_sc = number of 128-wide kv chunks."""
    fp32 = mybir.dt.float32
    bf16 = mybir.dt.bfloat16
    AF = mybir.ActivationFunctionType

    nc = bacc.Bacc("TRN2", target_bir_lowering=False, debug=False,
                   num_devices=N_CORES)

    qT_d = nc.dram_tensor("qT", [DIM, LC], bf16, kind="ExternalInput")
    kvT_d = nc.dram_tensor("kvT", [DIM, S], bf16, kind="ExternalInput")
    wq_d = nc.dram_tensor("wq", [DIM, DIM], bf16, kind="ExternalInput")
    wk_d = nc.dram_tensor("wk", [DIM, DIM], bf16, kind="ExternalInput")
    wv_d = nc.dram_tensor("wv", [DIM, DIM], bf16, kind="ExternalInput")
    wo_d = nc.dram_tensor("wo", [DIM, DIM], bf16, kind="ExternalInput")
    mask_d = nc.dram_tensor("mask", [128, 16], fp32, kind="ExternalInput")
    qw_d = nc.dram_tensor("qw", [128, 1], fp32, kind="ExternalInput")
    kw_d = nc.dram_tensor("kw", [128, 1], fp32, kind="ExternalInput")
    ind_d = nc.dram_tensor("ind", [KC, 128, 16], bf16, kind="ExternalInput")
    ind2_d = nc.dram_tensor("ind2", [KC, 16, 128], fp32, kind="ExternalInput")
    eye_d = nc.dram_tensor("eye16", [16, 16], fp32, kind="ExternalInput")
    out_d = nc.dram_tensor("out", [LC, DIM], fp32, kind="ExternalOutput")

    n_sg = (n_sc + 3) // 4           # 512-wide kv groups for the K projection
    SKC = n_sg * 4                   # padded chunk count for K-side tiles

    with tile.TileContext(nc) as tc:
        with (
            tc.tile_pool(name="wp", bufs=3 * KC) as wp,          # wq wk wv (+wo reuse)
            tc.tile_pool(name="qtp", bufs=KC) as qtp,            # qT, later oT
            tc.tile_pool(name="kvp", bufs=KC) as kvp,            # kvT (+sq/exp tiles)
            tc.tile_pool(name="qhp", bufs=KC) as qhp,            # qhT
            tc.tile_pool(name="khp", bufs=KC) as khp,            # khT
            tc.tile_pool(name="vp", bufs=n_sc) as vp,            # vh_aug
            tc.tile_pool(name="sp", bufs=1) as sp,               # constants
            tc.tile_pool(name="skp", bufs=SKC) as skp,           # skT per chunk
            tc.tile_pool(name="tp", bufs=4) as tp,               # f32 temps
            tc.tile_pool(name="pa", bufs=2, space="PSUM") as pa,     # proj/score psum
            tc.tile_pool(name="po", bufs=4, space="PSUM") as po,     # accum/bc psum
        ):
            # ---------------- constants / inputs ----------------
            wq_sb, wk_sb, wv_sb, kvt_sb, qt_sb = [], [], [], [], []
            for k in range(KC):
                w1 = wp.tile([128, DIM], bf16, name=f"wq{k}", tag="w")
                nc.sync.dma_start(out=w1, in_=wq_d[k * 128:(k + 1) * 128, :])
                wq_sb.append(w1)
            for k in range(KC):
                w2 = wp.tile([128, DIM], bf16, name=f"wk{k}", tag="w")
                nc.sync.dma_start(out=w2, in_=wk_d[k * 128:(k + 1) * 128, :])
                wk_sb.append(w2)
            for k in range(KC):
                w3 = wp.tile([128, DIM], bf16, name=f"wv{k}", tag="w")
                nc.sync.dma_start(out=w3, in_=wv_d[k * 128:(k + 1) * 128, :])
                wv_sb.append(w3)
            for k in range(KC):
                qt = qtp.tile([128, LC], bf16, name=f"qt{k}", tag="qt")
                nc.sync.dma_start(out=qt, in_=qT_d[k * 128:(k + 1) * 128, :])
                qt_sb.append(qt)
            for k in range(KC):
                kvt = kvp.tile([128, S], bf16, name=f"kvt{k}", tag="kv")
                nc.sync.dma_start(out=kvt, in_=kvT_d[k * 128:(k + 1) * 128, :])
                kvt_sb.append(kvt)

            mask_sb = sp.tile([128, 16], fp32, name="mask")
            nc.sync.dma_start(out=mask_sb, in_=mask_d[:, :])
            qw_sb = sp.tile([128, 1], fp32, name="qw")
            nc.sync.dma_start(out=qw_sb, in_=qw_d[:, :])
            kw_sb = sp.tile([128, 1], fp32, name="kw")
            nc.sync.dma_start(out=kw_sb, in_=kw_d[:, :])
            eye_sb = sp.tile([16, 16], fp32, name="eye16")
            nc.sync.dma_start(out=eye_sb, in_=eye_d[:, :])
            ind_sb, ind2_sb = [], []
            for k in range(KC):
                i1 = sp.tile([128, 16], bf16, name=f"ind{k}")
                nc.sync.dma_start(out=i1, in_=ind_d[k, :, :])
                ind_sb.append(i1)
                i2 = sp.tile([16, 128], fp32, name=f"ind2{k}")
                nc.sync.dma_start(out=i2, in_=ind2_d[k, :, :])
                ind2_sb.append(i2)
            epsq_sb = sp.tile([16, 1], fp32, name="epsq")
            nc.vector.memset(epsq_sb, EPS)
            epsk_sb = sp.tile([16, 1], fp32, name="epsk")
            nc.vector.memset(epsk_sb, 64.0 * EPS)

            # ---------------- Q projection + q RMS stats ----------------
            # sumsq matmuls are emitted one (m, j) step late so the PE
            # queue head never blocks on the ACT Square.
            qh_sb = [qhp.tile([128, LC], bf16, name=f"qh{m}", tag="qh")
                     for m in range(KC)]
            sumsq_q = [po.tile([16, 512], fp32, name=f"ssq{j}", tag="po")
                       for j in range(2)]
            pend = None                      # (m, j, qsq_tile)
            for m in range(KC):
                for j in range(2):
                    ps = pa.tile([128, 1024], fp32, name="proj_ps", tag="pa")
                    for k in range(KC):
                        nc.tensor.matmul(
                            ps[:, :512],
                            lhsT=wq_sb[k][:, m * 128:(m + 1) * 128],
                            rhs=qt_sb[k][:, j * 512:(j + 1) * 512],
                            start=(k == 0), stop=(k == KC - 1))
                    if pend is not None:
                        pm, pj, pq = pend
                        nc.tensor.matmul(
                            sumsq_q[pj][:, :], lhsT=ind_sb[pm],
                            rhs=pq[:, :512],
                            start=(pm == 0), stop=(pm == KC - 1))
                    nc.vector.tensor_scalar_mul(
                        qh_sb[m][:, j * 512:(j + 1) * 512], ps[:, :512], qw_sb)
                    qsq = kvp.tile([128, 1024], bf16, name="sqt", tag="sq",
                                   bufs=4)
                    nc.scalar.activation(qsq[:, :512], ps[:, :512], AF.Square)
                    pend = (m, j, qsq)
            pm, pj, pq = pend
            nc.tensor.matmul(sumsq_q[pj][:, :], lhsT=ind_sb[pm],
                             rhs=pq[:, :512], start=False, stop=True)
            # sq = 1/sqrt(mean + eps), broadcast to rows, fold into qhT
            sq_sb = []
            for j in range(2):
                sqr = tp.tile([16, 512], fp32, name=f"sqr{j}", tag="small16")
                nc.scalar.activation(sqr, sumsq_q[j][:, :], AF.Sqrt,
                                     scale=1.0 / HD, bias=epsq_sb)
                sqv = tp.tile([16, 512], fp32, name=f"sqv{j}", tag="small16b")
                nc.vector.reciprocal(out=sqv, in_=sqr)
                sq_sb.append(sqv)
            for m in range(KC):
                for j in range(2):
                    bc = po.tile([128, 512], fp32, name="qbc", tag="po")
                    nc.tensor.matmul(bc, lhsT=ind2_sb[m], rhs=sq_sb[j],
                                     start=True, stop=True)
                    nc.vector.tensor_mul(
                        qh_sb[m][:, j * 512:(j + 1) * 512],
                        qh_sb[m][:, j * 512:(j + 1) * 512], bc)

            # ---------------- K projection + skT ----------------
            kh_sb = [khp.tile([128, S], bf16, name=f"kh{m}", tag="kh")
                     for m in range(KC)]
            skT_sb = [skp.tile([128, 16], fp32, name=f"skT{i}", tag="sk")
                      for i in range(SKC)]
            for sg in range(n_sg):
                ssk = po.tile([16, 512], fp32, name="ssk", tag="po")
                pend = None
                for m in range(KC):
                    ps = pa.tile([128, 1024], fp32, name="proj_ps", tag="pa")
                    for k in range(KC):
                        nc.tensor.matmul(
                            ps[:, :512],
                            lhsT=wk_sb[k][:, m * 128:(m + 1) * 128],
                            rhs=kvt_sb[k][:, sg * 512:(sg + 1) * 512],
                            start=(k == 0), stop=(k == KC - 1))
                    if pend is not None:
                        pm, pq = pend
                        nc.tensor.matmul(
                            ssk[:, :], lhsT=ind_sb[pm], rhs=pq[:, :512],
                            start=(pm == 0), stop=False)
                    nc.vector.tensor_scalar_mul(
                        kh_sb[m][:, sg * 512:(sg + 1) * 512], ps[:, :512],
                        kw_sb)
                    ksq = kvp.tile([128, 1024], bf16, name="sqt", tag="sq",
                                   bufs=4)
                    nc.scalar.activation(ksq[:, :512], ps[:, :512], AF.Square)
                    pend = (m, ksq)
                pm, pq = pend
                nc.tensor.matmul(ssk[:, :], lhsT=ind_sb[pm], rhs=pq[:, :512],
                                 start=False, stop=True)
                # 8*sqrt(mean+eps) = sqrt(sumsq + 64 eps); recip -> sk/8
                skr = tp.tile([16, 512], fp32, name="skr", tag="small16")
                nc.scalar.activation(skr, ssk[:, :], AF.Sqrt,
                                     scale=1.0, bias=epsk_sb)
                for t in range(4):
                    tpp = po.tile([128, 16], fp32, name="sktp", tag="po")
                    nc.tensor.transpose(tpp, skr[:, t * 128:(t + 1) * 128],
                                        eye_sb)
                    nc.vector.reciprocal(out=skT_sb[sg * 4 + t], in_=tpp)

            # ---------------- V projection (token-major, ones-augmented) ----
            va_sb = []
            for i in range(n_sc):
                va = vp.tile([128, H * 65], bf16, name=f"va{i}", tag="va")
                ones_cols = bass.AP(tensor=va.tensor, offset=va.offset + 64,
                                    ap=[list(va.ap[0]), [65, H], [1, 1]])
                nc.vector.memset(ones_cols, 1.0)
                for jn in range(2):
                    ps = pa.tile([128, 1024], fp32, name="proj_ps", tag="pa")
                    for k in range(KC):
                        nc.tensor.matmul(
                            ps[:, :512],
                            lhsT=kvt_sb[k][:, i * 128:(i + 1) * 128],
                            rhs=wv_sb[k][:, jn * 512:(jn + 1) * 512],
                            start=(k == 0), stop=(k == KC - 1))
                    dst = bass.AP(tensor=va.tensor,
                                  offset=va.offset + 65 * 8 * jn,
                                  ap=[list(va.ap[0]), [65, 8], [1, 64]])
                    nc.vector.tensor_copy(
                        dst, ps[:, :512].rearrange("p (h d) -> p h d", h=8))
                va_sb.append(va)

            # ---------------- attention (head pairs, pipelined) ----------
            # oT holds UNNORMALIZED o^T; den_sb collects denominators.
            oT_sb = [qtp.tile([128, LC], bf16, name=f"oT{m}", tag="qt")
                     for m in range(KC)]
            den_sb = sp.tile([16, LC], fp32, name="den")
            for p in range(KC):              # head pair (2p, 2p+1)
                hA, hB = 2 * p, 2 * p + 1
                pv = [po.tile([128, 512], fp32, name=f"pv{x}", tag="po")
                      for x in range(4)]     # A0 A1 B0 B1
                pending = None               # (exA, exB, first)
                for i in range(n_sc):
                    scA = pa.tile([128, 1024], fp32, name="scA", tag="pa")
                    scB = pa.tile([128, 1024], fp32, name="scB", tag="pa")
                    for j in range(2):
                        nc.tensor.matmul(
                            scA[:, j * 512:(j + 1) * 512],
                            lhsT=kh_sb[p][0:64, i * 128:(i + 1) * 128],
                            rhs=qh_sb[p][0:64, j * 512:(j + 1) * 512],
                            start=True, stop=True, tile_position=(0, 0))
                        nc.tensor.matmul(
                            scB[:, j * 512:(j + 1) * 512],
                            lhsT=kh_sb[p][64:128, i * 128:(i + 1) * 128],
                            rhs=qh_sb[p][64:128, j * 512:(j + 1) * 512],
                            start=True, stop=True, tile_position=(64, 0))
                    if pending is not None:
                        exA, exB, first = pending
                        for j in range(2):
                            nc.tensor.matmul(
                                pv[j][:65, :],
                                lhsT=va_sb[i - 1][:, hA * 65:(hA + 1) * 65],
                                rhs=exA[:, j * 512:(j + 1) * 512],
                                start=first, stop=False)
                        for j in range(2):
                            nc.tensor.matmul(
                                pv[2 + j][:65, :],
                                lhsT=va_sb[i - 1][:, hB * 65:(hB + 1) * 65],
                                rhs=exB[:, j * 512:(j + 1) * 512],
                                start=first, stop=False)
                    exA = kvp.tile([128, 1024], bf16, name="exA", tag="sq",
                                   bufs=4)
                    exB = kvp.tile([128, 1024], bf16, name="exB", tag="sq",
                                   bufs=4)
                    nc.scalar.activation(exA, scA, AF.Exp,
                                         scale=skT_sb[i][:, hA:hA + 1],
                                         bias=mask_sb[:, i:i + 1])
                    nc.scalar.activation(exB, scB, AF.Exp,
                                         scale=skT_sb[i][:, hB:hB + 1],
                                         bias=mask_sb[:, i:i + 1])
                    pending = (exA, exB, i == 0)
                exA, exB, first = pending
                for j in range(2):
                    nc.tensor.matmul(
                        pv[j][:65, :],
                        lhsT=va_sb[n_sc - 1][:, hA * 65:(hA + 1) * 65],
                        rhs=exA[:, j * 512:(j + 1) * 512],
                        start=first, stop=True)
                for j in range(2):
                    nc.tensor.matmul(
                        pv[2 + j][:65, :],
                        lhsT=va_sb[n_sc - 1][:, hB * 65:(hB + 1) * 65],
                        rhs=exB[:, j * 512:(j + 1) * 512],
                        start=first, stop=True)
                # stash unnormalized o^T and the denominators (off the PE
                # path).  Engine ops need 32-aligned partition bases, so the
                # denominator row is staged at partition 64 and moved to its
                # per-head row in den_sb by a small SBUF->SBUF DMA.
                for x, (hh, j) in enumerate(((hA, 0), (hA, 1),
                                             (hB, 0), (hB, 1))):
                    poff = (hh % 2) * 64
                    nc.vector.tensor_copy(
                        oT_sb[p][poff:poff + 64, j * 512:(j + 1) * 512],
                        pv[x][0:64, :])
                    dstage = tp.tile([128, 512], fp32, name="dstage",
                                     tag="rec")
                    nc.vector.tensor_copy(dstage[64:65, :], pv[x][64:65, :])
                    nc.sync.dma_start(
                        out=den_sb[hh:hh + 1, j * 512:(j + 1) * 512],
                        in_=dstage[64:65, :])

            # ---------------- normalize o^T (batched) ----------------
            denr_sb = sp.tile([16, LC], fp32, name="denr")
            nc.vector.reciprocal(out=denr_sb, in_=den_sb)
            for m in range(KC):
                for j in range(2):
                    bc = po.tile([128, 512], fp32, name="obc", tag="po")
                    nc.tensor.matmul(bc, lhsT=ind2_sb[m],
                                     rhs=denr_sb[:, j * 512:(j + 1) * 512],
                                     start=True, stop=True)
                    nc.vector.tensor_mul(
                        oT_sb[m][:, j * 512:(j + 1) * 512],
                        oT_sb[m][:, j * 512:(j + 1) * 512], bc)

            # ---------------- output projection ----------------
            wo_sb = []
            for k in range(KC):
                w4 = wp.tile([128, DIM], bf16, name=f"wo{k}", tag="w")
                nc.sync.dma_start(out=w4, in_=wo_d[k * 128:(k + 1) * 128, :])
                wo_sb.append(w4)
            for lc in range(KC):
                for jn in range(2):
                    ps = pa.tile([128, 1024], fp32, name="proj_ps", tag="pa")
                    for k in range(KC):
                        nc.tensor.matmul(
                            ps[:, :512],
                            lhsT=oT_sb[k][:, lc * 128:(lc + 1) * 128],
                            rhs=wo_sb[k][:, jn * 512:(jn + 1) * 512],
                            start=(k == 0), stop=(k == KC - 1))
                    osb = tp.tile([128, 512], fp32, name="osb", tag="rec")
                    nc.vector.tensor_copy(osb, ps[:, :512])
                    nc.sync.dma_start(
                        out=out_d[lc * 128:(lc + 1) * 128,
                                  jn * 512:(jn + 1) * 512],
                        in_=osb)
    nc.compile()
    return nc


def kernel(**inputs):
    q = np.asarray(inputs["q"], dtype=np.float32)
    kv = np.asarray(inputs["kv"], dtype=np.float32)
    seqlens = np.asarray(inputs["x_seqlens"], dtype=np.int32)
    Wq = np.asarray(inputs["Wq"], dtype=np.float32)
    Wk = np.asarray(inputs["Wk"], dtype=np.float32)
    Wv = np.asarray(inputs["Wv"], dtype=np.float32)
    Wo = np.asarray(inputs["Wo"], dtype=np.float32)
    qnw = np.asarray(inputs["q_norm_w"], dtype=np.float32)
    knw = np.asarray(inputs["k_norm_w"], dtype=np.float32)

    n_sc = max(1, int(-(-int(seqlens.max()) // 128)))
    if n_sc not in _CACHE:
        _CACHE[n_sc] = _build(n_sc)
    nc = _CACHE[n_sc]

    wq_b = np.ascontiguousarray(Wq).astype(BF16)
    wk_b = np.ascontiguousarray(Wk).astype(BF16)
    wv_b = np.ascontiguousarray(Wv).astype(BF16)
    wo_b = np.ascontiguousarray(Wo).astype(BF16)
    qw = np.tile(qnw, 2).reshape(128, 1)
    kw = np.tile(knw, 2).reshape(128, 1)
    ind = np.zeros((KC, 128, 16), np.float32)
    ind2 = np.zeros((KC, 16, 128), np.float32)
    p = np.arange(128)
    for c in range(KC):
        ind[c, p, 2 * c + p // 64] = 1.0
        ind2[c, 2 * c + p // 64, p] = 1.0
    ind = ind.astype(BF16)
    eye16 = np.eye(16, dtype=np.float32)

    in_maps = []
    for c in range(N_CORES):
        b, half = c // 2, c % 2
        qT = np.ascontiguousarray(
            q[b, half * LC:(half + 1) * LC, :].T).astype(BF16)
        kvT = np.ascontiguousarray(kv[b].T).astype(BF16)
        sl = int(seqlens[b])
        mask = np.where(np.arange(S) < sl, 0.0, MASK_BIAS).astype(np.float32)
        mask = np.ascontiguousarray(mask.reshape(16, 128).T)
        in_maps.append({
            "qT": qT, "kvT": kvT, "wq": wq_b, "wk": wk_b, "wv": wv_b,
            "wo": wo_b, "mask": mask, "qw": qw, "kw": kw, "ind": ind,
            "ind2": ind2, "eye16": eye16,
        })

    res = run_bass_kernel_spmd(nc, in_maps, list(range(N_CORES)),
                               trace=TRACE)
    LAST_RESULT["exec_time_ns"] = res.exec_time_ns
    LAST_RESULT["profile"] = res.profile_json

    out = np.empty((B, L, DIM), np.float32)
    for c in range(N_CORES):
        b, half = c // 2, c % 2
        out[b, half * LC:(half + 1) * LC, :] = res.results[c]["out"]
    return out
